# revision 19
# baseline (speedup 1.0000x reference)
"""Trainium2 Bass kernel for windowed (banded) self-attention MLP block.

Reference computation (per batch b):
    h = relu(x @ W1 + b1)                      # [S, H]
    q = h @ Wq                                 # [S, H]
    scores[s, w] = q[s] . h_pad[s + w] / 32    # window w in [0, 33), h zero-padded by A=16
    wgt = softmax(scores, axis=w)
    out[s] = sum_w wgt[s, w] * h_pad[s + w]

Sharding: 8 cores, each takes 1024 consecutive tokens of the flattened
[B*S] = 8192 token stream (2 cores per batch element; shards never cross a
batch boundary).  Each core redundantly computes h for a 16-token halo on
each side, so no cross-core communication is needed.

v2 layout (per-core DRAM, host prepares):
    xa  [128, 4, 1152] bf16   x^T chunked along IN (zero-padded tokens)
    w1  [8, 128, 4, 128] bf16 W1 chunked hc-major
    wq8 [128, 2, 4, 1024] f8  (Wq * 2^10) pair-plane layout for DoubleRow
    b1c [128, 8] f32          b1 as per-hc bias columns
    hm  [128, 2] f32          halo validity multipliers (left, right)
    out [1024, 1024] bf16     (host casts back to f32)

On-chip stages (fp32 PSUM accumulation):
    A:  hT[hc, t] = relu(W1^T @ xT + b1)   H-on-partitions, 1056 tokens;
        weights reused across 3 token tiles (LDW amortization); bias+relu
        fused in one DVE tensor_scalar; halo cols zeroed via hm;
        hT8 = fp8(16*h) for the core 1024 tokens (gpsimd quantize)
    B:  qT[ho, t] = (Wq)^T @ hT via fp8 DoubleRow matmuls (K=256 per MM),
        scale 2^-19 folded into the PSUM->qT copy (qT = q/32 in bf16)
    T:  hh[t, hc] = hT^T via 72 PE transposes (token-major h for stage D)
    D:  per 128-token tile: scores = qT^T @ hT_window  [128, 160]
        p = exp(scores + bandmask) (bf16) + denominator via ACT accum_out,
        pT via PE transpose; out = (pT^T @ hh_window) * (1/den)
"""

import sys

import numpy as np

try:
    import concourse.bass as bass
except ImportError:
    sys.path.insert(0, "/opt/trn_rl_repo")
    import concourse.bass as bass

import ml_dtypes

import concourse.mybir as mybir
import concourse.tile as tile
from concourse import bacc
from concourse.bass_utils import run_bass_kernel_spmd

BF16 = ml_dtypes.bfloat16
F8 = ml_dtypes.float8_e4m3

B, S, IN, H = 4, 2048, 512, 1024
A = 16
WND = 2 * A + 1            # 33 window positions
NCORES = 8
TOK = (B * S) // NCORES    # 1024 tokens per core
TOKH = TOK + 2 * A         # 1056 with halo
TOKP = 9 * 128             # 1152 zero-padded token slots
NT = TOK // 128            # 8 output tiles per core
WIN = 128 + 2 * A          # 160-token window per 128-token tile
NEG = -30000.0             # additive mask for out-of-band positions

FP8_B = True               # stage B via fp8 DoubleRow (else bf16)
WQ_SCALE = 2.0 ** 10       # host-side Wq multiplier for fp8 range
H8_SCALE = 16.0            # hT8 = 16*h
QT_SCALE = 1.0 / (WQ_SCALE * H8_SCALE * 32.0)  # PSUM -> qT (q/32)

f32 = mybir.dt.float32
bf16 = mybir.dt.bfloat16
fp8 = mybir.dt.float8e4
AF = mybir.ActivationFunctionType
ALU = mybir.AluOpType
DR = mybir.MatmulPerfMode.DoubleRow


def _band_mask():
    """[128, WIN] additive mask: row t allows window cols t..t+32."""
    m = np.full((128, WIN), NEG, dtype=np.float32)
    for t in range(128):
        m[t, t : t + WND] = 0.0
    return m


def _kernel_body(tc, nc, xa_d, w1_d, wq_d, wq8_d, cst_d, out_d, id_d):
    with (
        tc.tile_pool(name="const", bufs=1) as cpool,
        tc.tile_pool(name="wts", bufs=1) as wpool,
        tc.tile_pool(name="acts", bufs=1) as apool,
    ):
        xa = wpool.tile([128, 4, TOKP], bf16, tag="xa")
        w1 = wpool.tile([128, 8, 4, 128], bf16, tag="w1")
        # b1c/hm/mask packed into one [128, 170] f32 tensor: DMA cost is
        # ~60ns per partition-row descriptor regardless of size, so small
        # tensors must share one transfer
        cst = cpool.tile([128, 170], f32, tag="cst")
        warm = wpool.tile([128, 512], bf16, tag="warm")
        nc.gpsimd.memset(warm[:], 0.0)
        # immediate-scalar tensor_scalar ops hit a ~30x slow path on DVE;
        # keep all scalars as [128,1] per-partition APs instead
        c16 = cpool.tile([128, 1], f32, tag="c16")
        nc.gpsimd.memset(c16[:], H8_SCALE)
        cqs = cpool.tile([128, 1], f32, tag="cqs")
        nc.gpsimd.memset(cqs[:], QT_SCALE if FP8_B else 1.0 / 32)

        # DMA issue order = first-needed order.  Per-queue DMA runs at
        # ~52 GB/s with a ~60ns/partition-row descriptor floor, and each
        # dma_start issue costs ~650ns on its sequencer, so big tensors are
        # split by content AND partition halves across both HWDGE
        # sequencers to land just ahead of their first consumer.
        id_sb = cpool.tile([128, 128], bf16, tag="ident")
        if FP8_B:
            wq8 = wpool.tile([128, 2, 4, H], fp8, tag="wq8")
            wqt, wqt_d = wq8, wq8_d
        else:
            wq = wpool.tile([128, 8, H], bf16, tag="wq")
            wqt, wqt_d = wq, wq_d
        QTRS = ((0, 32), (32, 64), (64, 96), (96, 128))
        for p0, p1 in QTRS:
            nc.sync.dma_start(w1[p0:p1, 0:2], w1_d[p0:p1, 0:2])
        for p0, p1 in QTRS:
            nc.scalar.dma_start(xa[p0:p1, 0], xa_d[p0:p1, 0])
        for p0, p1 in ((0, 64), (64, 128)):
            nc.sync.dma_start(xa[p0:p1, 1], xa_d[p0:p1, 1])
            nc.scalar.dma_start(cst[p0:p1], cst_d[p0:p1])
        for p0, p1 in ((0, 64), (64, 128)):
            nc.sync.dma_start(w1[p0:p1, 2:5], w1_d[p0:p1, 2:5])
            nc.scalar.dma_start(xa[p0:p1, 2], xa_d[p0:p1, 2])
        for p0, p1 in ((0, 64), (64, 128)):
            nc.sync.dma_start(w1[p0:p1, 5:8], w1_d[p0:p1, 5:8])
            nc.scalar.dma_start(xa[p0:p1, 3], xa_d[p0:p1, 3])
        nc.scalar.dma_start(id_sb[:], id_d[:])
        for p0, p1 in ((0, 64), (64, 128)):
            nc.sync.dma_start(wqt[p0:p1], wqt_d[p0:p1])

        hT = apool.tile([128, 8, TOKH], bf16, tag="hT")
        if FP8_B:
            hT8 = apool.tile([128, 8, TOK], fp8, tag="hT8")
        qT = apool.tile([128, 8, TOK], bf16, tag="qT")
        hh = apool.tile([128, 9, H], bf16, tag="hh")

        # ---- stage A: hT = relu(W1^T @ xT + b1) ----
        A_TILES = ((0, 512), (512, 1024), (1024, TOKH))
        with tc.tile_pool(name="psA", bufs=1, space="PSUM") as psA:
            # PE warm-up: matmuls on a zeroed scratch tile during the input
            # DMA wait release the HAM clock gate (2.4 GHz) before stage A.
            for _ in range(5):
                wps = psA.tile([128, 512], f32, tag="warm", bufs=1)
                nc.tensor.matmul(
                    wps[:], warm[:, 0:128], warm[:], start=True, stop=True
                )
            for hc in range(8):
                ps = [
                    psA.tile(
                        [128, t1 - t0], f32, tag=f"pa{i}", bufs=2, name=f"pa{i}"
                    )
                    for i, (t0, t1) in enumerate(A_TILES)
                ]
                # c outer / token-tile inner: each W1 chunk load feeds 3 MMs
                for c in range(4):
                    for i, (t0, t1) in enumerate(A_TILES):
                        nc.tensor.matmul(
                            ps[i][:],
                            w1[:, hc, c, :],
                            xa[:, c, t0:t1],
                            start=(c == 0),
                            stop=(c == 3),
                        )
                for i, (t0, t1) in enumerate(A_TILES):
                    nc.vector.tensor_scalar(
                        hT[:, hc, t0:t1],
                        ps[i][:],
                        cst[:, hc : hc + 1],
                        0.0,
                        ALU.add,
                        ALU.max,
                    )
                # zero halo cols outside this core's batch, then quantize
                nc.vector.tensor_scalar_mul(
                    hT[:, hc, 0:A], hT[:, hc, 0:A], cst[:, 8:9]
                )
                nc.vector.tensor_scalar_mul(
                    hT[:, hc, TOK + A : TOKH], hT[:, hc, TOK + A : TOKH],
                    cst[:, 9:10],
                )
                if FP8_B:
                    # ACT is idle during stage A; native scale path is fast
                    nc.scalar.activation(
                        hT8[:, hc, :], hT[:, hc, A : A + TOK], AF.Copy,
                        scale=H8_SCALE,
                    )

        # ---- stage T (hh transposes) + stage B (qT) ----
        with tc.tile_pool(name="psBT", bufs=1, space="PSUM") as psBT:
            # hh transposes: hh[:, t, hc*128:...] = hT[:, hc, t*128:...]^T
            # 8 full tiles + the 32-token tail (tokens 1024:1056)
            eng = 0
            for t in range(9):
                for hc in range(8):
                    pt = psBT.tile([128, 128], bf16, tag="pt", bufs=4)
                    osl = slice(hc * 128, (hc + 1) * 128)
                    if t < 8:
                        nc.tensor.transpose(
                            pt[:], hT[:, hc, t * 128 : (t + 1) * 128], id_sb[:]
                        )
                        src = pt[:]
                        dst = hh[:, t, osl]
                    else:
                        nc.tensor.transpose(
                            pt[0:32, :], hT[:, hc, 1024:TOKH], id_sb[:]
                        )
                        src = pt[0:32, :]
                        dst = hh[0:32, t, osl]
                    # PSUM is only readable from DVE/ACT; alternate them
                    if eng == 0:
                        nc.vector.tensor_copy(dst, src)
                    else:
                        nc.scalar.copy(dst, src)
                    eng = (eng + 1) % 2

            for ho in range(8):
                osl = slice(ho * 128, (ho + 1) * 128)
                q0 = psBT.tile([128, 512], f32, tag="q0", bufs=2)
                q1 = psBT.tile([128, 512], f32, tag="q1", bufs=2)
                if FP8_B:
                    for c in range(4):
                        nc.tensor.matmul(
                            q0[:], wq8[:, :, c, osl], hT8[:, 2 * c : 2 * c + 2, 0:512],
                            start=(c == 0), stop=(c == 3), perf_mode=DR,
                        )
                        nc.tensor.matmul(
                            q1[:], wq8[:, :, c, osl], hT8[:, 2 * c : 2 * c + 2, 512:1024],
                            start=(c == 0), stop=(c == 3), perf_mode=DR,
                        )
                else:
                    for hi in range(8):
                        nc.tensor.matmul(
                            q0[:], wq[:, hi, osl], hT[:, hi, A : A + 512],
                            start=(hi == 0), stop=(hi == 7),
                        )
                        nc.tensor.matmul(
                            q1[:], wq[:, hi, osl], hT[:, hi, A + 512 : A + 1024],
                            start=(hi == 0), stop=(hi == 7),
                        )
                qsc = QT_SCALE if FP8_B else 1.0 / 32
                if ho == 7:
                    # split the final copies across both engines so the
                    # psBT pool drains (and psD opens) sooner
                    nc.vector.tensor_scalar_mul(qT[:, ho, 0:256], q0[:, 0:256], cqs[:, 0:1])
                    nc.scalar.activation(qT[:, ho, 256:512], q0[:, 256:512], AF.Copy, scale=qsc)
                    nc.vector.tensor_scalar_mul(qT[:, ho, 512:768], q1[:, 0:256], cqs[:, 0:1])
                    nc.scalar.activation(qT[:, ho, 768:1024], q1[:, 256:512], AF.Copy, scale=qsc)
                elif ho % 2 == 0:
                    nc.vector.tensor_scalar_mul(qT[:, ho, 0:512], q0[:], cqs[:, 0:1])
                    nc.scalar.activation(qT[:, ho, 512:1024], q1[:], AF.Copy, scale=qsc)
                else:
                    nc.scalar.activation(qT[:, ho, 0:512], q0[:], AF.Copy, scale=qsc)
                    nc.vector.tensor_scalar_mul(qT[:, ho, 512:1024], q1[:], cqs[:, 0:1])

        # ---- stage D: windowed attention per 128-token tile ----
        with (
            tc.tile_pool(name="psD", bufs=1, space="PSUM") as psD,
            tc.tile_pool(name="dtmp", bufs=2) as dpool,
            tc.tile_pool(name="outp", bufs=4) as opool,
        ):
            for T in range(NT):
                ps_s = psD.tile([128, WIN], f32, tag="ps", bufs=2)
                for hc in range(8):
                    nc.tensor.matmul(
                        ps_s[:],
                        qT[:, hc, T * 128 : (T + 1) * 128],
                        hT[:, hc, T * 128 : T * 128 + WIN],
                        start=(hc == 0),
                        stop=(hc == 7),
                    )
                s_sb = dpool.tile([128, WIN], f32, tag="s")
                nc.vector.tensor_add(s_sb[:], ps_s[:], cst[:, 10:170])
                p_sb = dpool.tile([128, WIN], bf16, tag="p")
                den = dpool.tile([128, 1], f32, tag="den")
                nc.scalar.activation(p_sb[:], s_sb[:], AF.Exp, accum_out=den[:])
                rcp = dpool.tile([128, 1], f32, tag="rcp")
                nc.vector.reciprocal(rcp[:], den[:])

                ptm = psD.tile([128, 256], bf16, tag="ptp", bufs=2)
                nc.tensor.transpose(ptm[:, 0:128], p_sb[:, 0:128], id_sb[:])
                nc.tensor.transpose(ptm[0:32, 128:256], p_sb[:, 128:WIN], id_sb[:])
                pta_sb = dpool.tile([128, 256], bf16, tag="pta")
                nc.vector.tensor_copy(pta_sb[:, 0:128], ptm[:, 0:128])
                nc.vector.tensor_copy(pta_sb[0:32, 128:256], ptm[0:32, 128:256])

                out_sb = opool.tile([128, H], bf16, tag="osb")
                pav0 = psD.tile([128, 512], f32, tag="pav0", bufs=2)
                pav1 = psD.tile([128, 512], f32, tag="pav1", bufs=2)
                # group by stationary operand: 2 LDWs per tile instead of 4
                nc.tensor.matmul(
                    pav0[:], pta_sb[:, 0:128], hh[:, T, 0:512],
                    start=True, stop=False,
                )
                nc.tensor.matmul(
                    pav1[:], pta_sb[:, 0:128], hh[:, T, 512:1024],
                    start=True, stop=False,
                )
                nc.tensor.matmul(
                    pav0[:], pta_sb[0:32, 128:256], hh[0:32, T + 1, 0:512],
                    start=False, stop=True,
                )
                nc.tensor.matmul(
                    pav1[:], pta_sb[0:32, 128:256], hh[0:32, T + 1, 512:1024],
                    start=False, stop=True,
                )
                if T < NT - 1:
                    nc.vector.tensor_scalar_mul(out_sb[:, 0:512], pav0[:], rcp[:])
                    nc.scalar.mul(out_sb[:, 512:1024], pav1[:], rcp[:])
                    for p0, p1 in ((0, 64), (64, 128)):
                        nc.sync.dma_start(
                            out_d[T * 128 + p0 : T * 128 + p1, :],
                            out_sb[p0:p1, :],
                        )
                else:
                    # last tile: quarter the DMA across both sequencers to
                    # shorten the end-of-kernel transfer tail
                    nc.vector.tensor_scalar_mul(out_sb[:, 0:512], pav0[:], rcp[:])
                    nc.scalar.mul(out_sb[:, 512:1024], pav1[:], rcp[:])
                    for i, (p0, p1) in enumerate(
                        ((0, 32), (32, 64), (64, 96), (96, 128))
                    ):
                        eng_d = nc.sync if i % 2 == 0 else nc.scalar
                        eng_d.dma_start(
                            out_d[T * 128 + p0 : T * 128 + p1, :],
                            out_sb[p0:p1, :],
                        )


def build_nc():
    nc = bacc.Bacc("TRN2", target_bir_lowering=False, debug=False, num_devices=NCORES)
    xa_d = nc.dram_tensor("xa", [128, 4, TOKP], bf16, kind="ExternalInput")
    w1_d = nc.dram_tensor("w1", [128, 8, 4, 128], bf16, kind="ExternalInput")
    if FP8_B:
        wq8_d = nc.dram_tensor("wq8", [128, 2, 4, H], fp8, kind="ExternalInput")
        wq_d = None
    else:
        wq_d = nc.dram_tensor("wq", [128, 8, H], bf16, kind="ExternalInput")
        wq8_d = None
    cst_d = nc.dram_tensor("cst", [128, 170], f32, kind="ExternalInput")
    out_d = nc.dram_tensor("out", [TOK, H], bf16, kind="ExternalOutput")
    id_d = nc.inline_tensor(np.eye(128, dtype=BF16), "ident")

    with tile.TileContext(nc) as tc:
        _kernel_body(tc, nc, xa_d, w1_d, wq_d, wq8_d, cst_d, out_d, id_d)
    nc.compile()
    return nc


def make_inputs(x, W1, b1, Wq):
    """Host-side shard prep (numpy only; not part of HW time)."""
    x = np.asarray(x, dtype=np.float32)
    W1 = np.asarray(W1, dtype=np.float32)
    b1 = np.asarray(b1, dtype=np.float32)
    Wq = np.asarray(Wq, dtype=np.float32)

    # w1a[p, hc, c, j] = W1[c*128 + p, hc*128 + j]  (p-major for fat DMA rows)
    w1a = np.ascontiguousarray(
        W1.reshape(4, 128, 8, 128).transpose(1, 2, 0, 3)
    ).astype(BF16)

    b1c = np.ascontiguousarray(b1.reshape(8, 128).T).astype(np.float32)  # [128, 8]

    if FP8_B:
        # wq8[p, i, c, k] = (Wq * WQ_SCALE)[(2c+i)*128 + p, k]
        wq8 = np.ascontiguousarray(
            (Wq * WQ_SCALE).reshape(4, 2, 128, H).transpose(2, 1, 0, 3)
        ).astype(F8)
        wq_common = {"wq8": wq8}
    else:
        wqs = (Wq).astype(BF16)
        wqa = np.zeros((128, 8, H), dtype=BF16)
        for c in range(8):
            wqa[:, c, :] = wqs[c * 128 : (c + 1) * 128, :]
        wq_common = {"wq": wqa}

    cstv = np.zeros((128, 170), dtype=np.float32)
    cstv[:, 0:8] = b1c
    cstv[:, 10:170] = _band_mask()

    in_maps = []
    for core in range(NCORES):
        b, half = divmod(core, 2)
        lo = half * TOK - A
        hi = half * TOK + TOK + A
        s0, s1 = max(lo, 0), min(hi, S)
        xs = np.zeros((TOKP, IN), dtype=np.float32)
        xs[s0 - lo : s1 - lo] = x[b, s0:s1]
        xT = np.ascontiguousarray(xs.T).astype(BF16)  # [512, 1152]
        xa = np.ascontiguousarray(xT.reshape(4, 128, TOKP).transpose(1, 0, 2))
        cstc = cstv.copy()
        cstc[:, 8] = 1.0 if lo >= 0 else 0.0
        cstc[:, 9] = 1.0 if hi <= S else 0.0
        in_maps.append({"xa": xa, "w1": w1a, "cst": cstc, **wq_common})
    return in_maps


_NC_CACHE = {}


def get_nc():
    if "nc" not in _NC_CACHE:
        _NC_CACHE["nc"] = build_nc()
    return _NC_CACHE["nc"]


def kernel(x, W1, b1, Wq, atten_size, _trace=False, _trace_kwargs=None):
    assert int(atten_size) == A, f"kernel hardcodes atten_size=16, got {atten_size}"
    nc = get_nc()
    in_maps = make_inputs(x, W1, b1, Wq)
    kw = {}
    if _trace:
        kw = dict(trace=True, trace_kwargs=_trace_kwargs or {})
    res = run_bass_kernel_spmd(nc, in_maps, core_ids=list(range(NCORES)), **kw)
    out = np.stack([r["out"].astype(np.float32) for r in res.results])
    out = out.reshape(B, S, H)
    if _trace:
        return out, res
    return out


if __name__ == "__main__":
    import jax

    key = jax.random.key(0)
    k1, k2, k3, k4 = jax.random.split(key, 4)
    x = np.asarray(jax.random.normal(k1, (B, S, IN), dtype=np.float32))
    W1 = np.asarray(
        jax.random.normal(k2, (IN, H), dtype=np.float32) * (1.0 / np.sqrt(IN))
    )
    b1 = np.asarray(jax.random.normal(k3, (H,), dtype=np.float32) * 0.02)
    Wq = np.asarray(
        jax.random.normal(k4, (H, H), dtype=np.float32) * (1.0 / np.sqrt(H))
    )
    out = kernel(x, W1, b1, Wq, 16)
    print("out", out.shape, out.dtype, float(np.abs(out).max()))


# revision 20
# speedup vs baseline: 1.1328x; 1.1328x over previous
"""Trainium2 Bass kernel for windowed (banded) self-attention MLP block.

Reference computation (per batch b):
    h = relu(x @ W1 + b1)                      # [S, H]
    q = h @ Wq                                 # [S, H]
    scores[s, w] = q[s] . h_pad[s + w] / 32    # window w in [0, 33), h zero-padded by A=16
    wgt = softmax(scores, axis=w)
    out[s] = sum_w wgt[s, w] * h_pad[s + w]

Sharding: 8 cores, each takes 1024 consecutive tokens of the flattened
[B*S] = 8192 token stream (2 cores per batch element; shards never cross a
batch boundary).  Each core redundantly computes h for a 16-token halo on
each side, so no cross-core communication is needed.

v2 layout (per-core DRAM, host prepares):
    xa  [128, 4, 1152] bf16   x^T chunked along IN (zero-padded tokens)
    w1  [8, 128, 4, 128] bf16 W1 chunked hc-major
    wq8 [128, 2, 4, 1024] f8  (Wq * 2^10) pair-plane layout for DoubleRow
    b1c [128, 8] f32          b1 as per-hc bias columns
    hm  [128, 2] f32          halo validity multipliers (left, right)
    out [1024, 1024] bf16     (host casts back to f32)

On-chip stages (fp32 PSUM accumulation):
    A:  hT[hc, t] = relu(W1^T @ xT + b1)   H-on-partitions, 1056 tokens;
        weights reused across 3 token tiles (LDW amortization); bias+relu
        fused in one DVE tensor_scalar; halo cols zeroed via hm;
        hT8 = fp8(16*h) for the core 1024 tokens (gpsimd quantize)
    B:  qT[ho, t] = (Wq)^T @ hT via fp8 DoubleRow matmuls (K=256 per MM),
        scale 2^-19 folded into the PSUM->qT copy (qT = q/32 in bf16)
    T:  hh[t, hc] = hT^T via 72 PE transposes (token-major h for stage D)
    D:  per 128-token tile: scores = qT^T @ hT_window  [128, 160]
        p = exp(scores + bandmask) (bf16) + denominator via ACT accum_out,
        pT via PE transpose; out = (pT^T @ hh_window) * (1/den)
"""

import sys

import numpy as np

try:
    import concourse.bass as bass
except ImportError:
    sys.path.insert(0, "/opt/trn_rl_repo")
    import concourse.bass as bass

import ml_dtypes

import concourse.mybir as mybir
import concourse.tile as tile
from concourse import bacc
from concourse.bass_utils import run_bass_kernel_spmd

BF16 = ml_dtypes.bfloat16
F8 = ml_dtypes.float8_e4m3

B, S, IN, H = 4, 2048, 512, 1024
A = 16
WND = 2 * A + 1            # 33 window positions
NCORES = 8
TOK = (B * S) // NCORES    # 1024 tokens per core
TOKH = TOK + 2 * A         # 1056 with halo
TOKP = 9 * 128             # 1152 zero-padded token slots
NT = TOK // 128            # 8 output tiles per core
WIN = 128 + 2 * A          # 160-token window per 128-token tile
NEG = -30000.0             # additive mask for out-of-band positions

FP8_B = True               # stage B via fp8 DoubleRow (else bf16)
WQ_SCALE = 2.0 ** 10       # host-side Wq multiplier for fp8 range
H8_SCALE = 16.0            # hT8 = 16*h
QT_SCALE = 1.0 / (WQ_SCALE * H8_SCALE * 32.0)  # PSUM -> qT (q/32)

f32 = mybir.dt.float32
bf16 = mybir.dt.bfloat16
fp8 = mybir.dt.float8e4
AF = mybir.ActivationFunctionType
ALU = mybir.AluOpType
DR = mybir.MatmulPerfMode.DoubleRow


def _band_mask():
    """[128, WIN] additive mask: row t allows window cols t..t+32."""
    m = np.full((128, WIN), NEG, dtype=np.float32)
    for t in range(128):
        m[t, t : t + WND] = 0.0
    return m


def _kernel_body(tc, nc, xa_d, w1_d, wq_d, wq8_d, cst_d, out_d, id_d):
    with (
        tc.tile_pool(name="const", bufs=1) as cpool,
        tc.tile_pool(name="wts", bufs=1) as wpool,
        tc.tile_pool(name="acts", bufs=1) as apool,
    ):
        xa = wpool.tile([128, 4, TOKP], bf16, tag="xa")
        w1 = wpool.tile([128, 8, 4, 128], bf16, tag="w1")
        # b1c/hm/mask packed into one [128, 170] f32 tensor: DMA cost is
        # ~60ns per partition-row descriptor regardless of size, so small
        # tensors must share one transfer
        cst = cpool.tile([128, 170], f32, tag="cst")
        warm = wpool.tile([128, 512], bf16, tag="warm")
        nc.gpsimd.memset(warm[:], 0.0)
        # immediate-scalar tensor_scalar ops hit a ~30x slow path on DVE;
        # keep all scalars as [128,1] per-partition APs instead
        c16 = cpool.tile([128, 1], f32, tag="c16")
        nc.gpsimd.memset(c16[:], H8_SCALE)
        cqs = cpool.tile([128, 1], f32, tag="cqs")
        nc.gpsimd.memset(cqs[:], QT_SCALE if FP8_B else 1.0 / 32)

        # DMA issue order = first-needed order.  Per-queue DMA runs at
        # ~52 GB/s with a ~60ns/partition-row descriptor floor, and each
        # dma_start issue costs ~650ns on its sequencer, so big tensors are
        # split by content AND partition halves across both HWDGE
        # sequencers to land just ahead of their first consumer.
        id_sb = cpool.tile([128, 128], bf16, tag="ident")
        if FP8_B:
            wq8 = wpool.tile([128, 2, 4, H], fp8, tag="wq8")
            wqt, wqt_d = wq8, wq8_d
        else:
            wq = wpool.tile([128, 8, H], bf16, tag="wq")
            wqt, wqt_d = wq, wq_d
        QTRS = ((0, 32), (32, 64), (64, 96), (96, 128))
        for p0, p1 in QTRS:
            nc.sync.dma_start(w1[p0:p1, 0:2], w1_d[p0:p1, 0:2])
        for p0, p1 in QTRS:
            nc.scalar.dma_start(xa[p0:p1, 0], xa_d[p0:p1, 0])
        for p0, p1 in ((0, 64), (64, 128)):
            nc.sync.dma_start(xa[p0:p1, 1], xa_d[p0:p1, 1])
            nc.scalar.dma_start(cst[p0:p1], cst_d[p0:p1])
        for p0, p1 in ((0, 64), (64, 128)):
            nc.sync.dma_start(w1[p0:p1, 2:5], w1_d[p0:p1, 2:5])
            nc.scalar.dma_start(xa[p0:p1, 2], xa_d[p0:p1, 2])
        for p0, p1 in ((0, 64), (64, 128)):
            nc.sync.dma_start(w1[p0:p1, 5:8], w1_d[p0:p1, 5:8])
            nc.scalar.dma_start(xa[p0:p1, 3], xa_d[p0:p1, 3])
        nc.scalar.dma_start(id_sb[:], id_d[:])
        for p0, p1 in ((0, 64), (64, 128)):
            nc.sync.dma_start(wqt[p0:p1], wqt_d[p0:p1])

        hT = apool.tile([128, 8, TOKH], bf16, tag="hT")
        if FP8_B:
            hT8 = apool.tile([128, 8, TOK], fp8, tag="hT8")
        qT = apool.tile([128, 8, TOK], bf16, tag="qT")
        hh = apool.tile([128, 9, H], bf16, tag="hh")

        # ---- stage A: hT = relu(W1^T @ xT + b1) ----
        A_TILES = ((0, 512), (512, 1024), (1024, TOKH))
        with tc.tile_pool(name="psA", bufs=1, space="PSUM") as psA:
            # PE warm-up: matmuls on a zeroed scratch tile during the input
            # DMA wait release the HAM clock gate (2.4 GHz) before stage A.
            for _ in range(6):
                wps = psA.tile([128, 512], f32, tag="warm", bufs=1)
                nc.tensor.matmul(
                    wps[:], warm[:, 0:128], warm[:], start=True, stop=True
                )
            for hc in range(8):
                ps = [
                    psA.tile(
                        [128, t1 - t0], f32, tag=f"pa{i}", bufs=2, name=f"pa{i}"
                    )
                    for i, (t0, t1) in enumerate(A_TILES)
                ]
                # c outer / token-tile inner: each W1 chunk load feeds 3 MMs
                for c in range(4):
                    for i, (t0, t1) in enumerate(A_TILES):
                        nc.tensor.matmul(
                            ps[i][:],
                            w1[:, hc, c, :],
                            xa[:, c, t0:t1],
                            start=(c == 0),
                            stop=(c == 3),
                        )
                for i, (t0, t1) in enumerate(A_TILES):
                    nc.vector.tensor_scalar(
                        hT[:, hc, t0:t1],
                        ps[i][:],
                        cst[:, hc : hc + 1],
                        0.0,
                        ALU.add,
                        ALU.max,
                    )
                # zero halo cols outside this core's batch, then quantize
                nc.vector.tensor_scalar_mul(
                    hT[:, hc, 0:A], hT[:, hc, 0:A], cst[:, 8:9]
                )
                nc.vector.tensor_scalar_mul(
                    hT[:, hc, TOK + A : TOKH], hT[:, hc, TOK + A : TOKH],
                    cst[:, 9:10],
                )
                if FP8_B:
                    # ACT is idle during stage A; native scale path is fast
                    nc.scalar.activation(
                        hT8[:, hc, :], hT[:, hc, A : A + TOK], AF.Copy,
                        scale=H8_SCALE,
                    )

        # ---- stage T (hh transposes) + stage B (qT) ----
        with tc.tile_pool(name="psBT", bufs=1, space="PSUM") as psBT:
            # hh transposes: hh[:, t, hc*128:...] = hT[:, hc, t*128:...]^T
            # 8 full tiles + the 32-token tail (tokens 1024:1056)
            eng = 0
            for t in range(9):
                for hc in range(8):
                    pt = psBT.tile([128, 128], bf16, tag="pt", bufs=4)
                    osl = slice(hc * 128, (hc + 1) * 128)
                    if t < 8:
                        nc.tensor.transpose(
                            pt[:], hT[:, hc, t * 128 : (t + 1) * 128], id_sb[:]
                        )
                        src = pt[:]
                        dst = hh[:, t, osl]
                    else:
                        nc.tensor.transpose(
                            pt[0:32, :], hT[:, hc, 1024:TOKH], id_sb[:]
                        )
                        src = pt[0:32, :]
                        dst = hh[0:32, t, osl]
                    # PSUM is only readable from DVE/ACT; alternate them
                    if eng == 0:
                        nc.vector.tensor_copy(dst, src)
                    else:
                        nc.scalar.copy(dst, src)
                    eng = (eng + 1) % 2

            for ho in range(8):
                osl = slice(ho * 128, (ho + 1) * 128)
                q0 = psBT.tile([128, 512], f32, tag="q0", bufs=2)
                q1 = psBT.tile([128, 512], f32, tag="q1", bufs=2)
                if FP8_B:
                    for c in range(4):
                        nc.tensor.matmul(
                            q0[:], wq8[:, :, c, osl], hT8[:, 2 * c : 2 * c + 2, 0:512],
                            start=(c == 0), stop=(c == 3), perf_mode=DR,
                        )
                        nc.tensor.matmul(
                            q1[:], wq8[:, :, c, osl], hT8[:, 2 * c : 2 * c + 2, 512:1024],
                            start=(c == 0), stop=(c == 3), perf_mode=DR,
                        )
                else:
                    for hi in range(8):
                        nc.tensor.matmul(
                            q0[:], wq[:, hi, osl], hT[:, hi, A : A + 512],
                            start=(hi == 0), stop=(hi == 7),
                        )
                        nc.tensor.matmul(
                            q1[:], wq[:, hi, osl], hT[:, hi, A + 512 : A + 1024],
                            start=(hi == 0), stop=(hi == 7),
                        )
                qsc = QT_SCALE if FP8_B else 1.0 / 32
                if ho % 2 == 0:
                    nc.vector.tensor_scalar_mul(qT[:, ho, 0:512], q0[:], cqs[:, 0:1])
                    nc.scalar.activation(qT[:, ho, 512:1024], q1[:], AF.Copy, scale=qsc)
                else:
                    nc.scalar.activation(qT[:, ho, 0:512], q0[:], AF.Copy, scale=qsc)
                    nc.vector.tensor_scalar_mul(qT[:, ho, 512:1024], q1[:], cqs[:, 0:1])

        # ---- stage D: windowed attention per 128-token tile ----
        with (
            tc.tile_pool(name="psD", bufs=1, space="PSUM") as psD,
            tc.tile_pool(name="dtmp", bufs=2) as dpool,
            tc.tile_pool(name="outp", bufs=3) as opool,
        ):
            for T in range(NT):
                ps_s = psD.tile([128, WIN], f32, tag="ps", bufs=2)
                for hc in range(8):
                    nc.tensor.matmul(
                        ps_s[:],
                        qT[:, hc, T * 128 : (T + 1) * 128],
                        hT[:, hc, T * 128 : T * 128 + WIN],
                        start=(hc == 0),
                        stop=(hc == 7),
                    )
                s_sb = dpool.tile([128, WIN], f32, tag="s")
                nc.vector.tensor_add(s_sb[:], ps_s[:], cst[:, 10:170])
                p_sb = dpool.tile([128, WIN], bf16, tag="p")
                den = dpool.tile([128, 1], f32, tag="den")
                nc.scalar.activation(p_sb[:], s_sb[:], AF.Exp, accum_out=den[:])
                rcp = dpool.tile([128, 1], f32, tag="rcp")
                nc.vector.reciprocal(rcp[:], den[:])

                ptm = psD.tile([128, 256], bf16, tag="ptp", bufs=2)
                nc.tensor.transpose(ptm[:, 0:128], p_sb[:, 0:128], id_sb[:])
                nc.tensor.transpose(ptm[0:32, 128:256], p_sb[:, 128:WIN], id_sb[:])
                pta_sb = dpool.tile([128, 256], bf16, tag="pta")
                nc.vector.tensor_copy(pta_sb[:, 0:128], ptm[:, 0:128])
                nc.vector.tensor_copy(pta_sb[0:32, 128:256], ptm[0:32, 128:256])

                out_sb = opool.tile([128, H], bf16, tag="osb")
                pav0 = psD.tile([128, 512], f32, tag="pav0", bufs=2)
                pav1 = psD.tile([128, 512], f32, tag="pav1", bufs=2)
                # group by stationary operand: 2 LDWs per tile instead of 4
                nc.tensor.matmul(
                    pav0[:], pta_sb[:, 0:128], hh[:, T, 0:512],
                    start=True, stop=False,
                )
                nc.tensor.matmul(
                    pav1[:], pta_sb[:, 0:128], hh[:, T, 512:1024],
                    start=True, stop=False,
                )
                nc.tensor.matmul(
                    pav0[:], pta_sb[0:32, 128:256], hh[0:32, T + 1, 0:512],
                    start=False, stop=True,
                )
                nc.tensor.matmul(
                    pav1[:], pta_sb[0:32, 128:256], hh[0:32, T + 1, 512:1024],
                    start=False, stop=True,
                )
                if T < NT - 1:
                    nc.vector.tensor_scalar_mul(out_sb[:, 0:512], pav0[:], rcp[:])
                    nc.scalar.mul(out_sb[:, 512:1024], pav1[:], rcp[:])
                    for p0, p1 in ((0, 64), (64, 128)):
                        nc.sync.dma_start(
                            out_d[T * 128 + p0 : T * 128 + p1, :],
                            out_sb[p0:p1, :],
                        )
                else:
                    # last tile: quarter the DMA across both sequencers to
                    # shorten the end-of-kernel transfer tail
                    nc.vector.tensor_scalar_mul(out_sb[:, 0:512], pav0[:], rcp[:])
                    nc.scalar.mul(out_sb[:, 512:1024], pav1[:], rcp[:])
                    for i, (p0, p1) in enumerate(
                        ((0, 32), (32, 64), (64, 96), (96, 128))
                    ):
                        eng_d = nc.sync if i % 2 == 0 else nc.scalar
                        eng_d.dma_start(
                            out_d[T * 128 + p0 : T * 128 + p1, :],
                            out_sb[p0:p1, :],
                        )


def build_nc():
    nc = bacc.Bacc("TRN2", target_bir_lowering=False, debug=False, num_devices=NCORES)
    xa_d = nc.dram_tensor("xa", [128, 4, TOKP], bf16, kind="ExternalInput")
    w1_d = nc.dram_tensor("w1", [128, 8, 4, 128], bf16, kind="ExternalInput")
    if FP8_B:
        wq8_d = nc.dram_tensor("wq8", [128, 2, 4, H], fp8, kind="ExternalInput")
        wq_d = None
    else:
        wq_d = nc.dram_tensor("wq", [128, 8, H], bf16, kind="ExternalInput")
        wq8_d = None
    cst_d = nc.dram_tensor("cst", [128, 170], f32, kind="ExternalInput")
    out_d = nc.dram_tensor("out", [TOK, H], bf16, kind="ExternalOutput")
    id_d = nc.inline_tensor(np.eye(128, dtype=BF16), "ident")

    with tile.TileContext(nc) as tc:
        _kernel_body(tc, nc, xa_d, w1_d, wq_d, wq8_d, cst_d, out_d, id_d)
    nc.compile()
    return nc


def make_inputs(x, W1, b1, Wq):
    """Host-side shard prep (numpy only; not part of HW time)."""
    x = np.asarray(x, dtype=np.float32)
    W1 = np.asarray(W1, dtype=np.float32)
    b1 = np.asarray(b1, dtype=np.float32)
    Wq = np.asarray(Wq, dtype=np.float32)

    # w1a[p, hc, c, j] = W1[c*128 + p, hc*128 + j]  (p-major for fat DMA rows)
    w1a = np.ascontiguousarray(
        W1.reshape(4, 128, 8, 128).transpose(1, 2, 0, 3)
    ).astype(BF16)

    b1c = np.ascontiguousarray(b1.reshape(8, 128).T).astype(np.float32)  # [128, 8]

    if FP8_B:
        # wq8[p, i, c, k] = (Wq * WQ_SCALE)[(2c+i)*128 + p, k]
        wq8 = np.ascontiguousarray(
            (Wq * WQ_SCALE).reshape(4, 2, 128, H).transpose(2, 1, 0, 3)
        ).astype(F8)
        wq_common = {"wq8": wq8}
    else:
        wqs = (Wq).astype(BF16)
        wqa = np.zeros((128, 8, H), dtype=BF16)
        for c in range(8):
            wqa[:, c, :] = wqs[c * 128 : (c + 1) * 128, :]
        wq_common = {"wq": wqa}

    cstv = np.zeros((128, 170), dtype=np.float32)
    cstv[:, 0:8] = b1c
    cstv[:, 10:170] = _band_mask()

    in_maps = []
    for core in range(NCORES):
        b, half = divmod(core, 2)
        lo = half * TOK - A
        hi = half * TOK + TOK + A
        s0, s1 = max(lo, 0), min(hi, S)
        xs = np.zeros((TOKP, IN), dtype=np.float32)
        xs[s0 - lo : s1 - lo] = x[b, s0:s1]
        xT = np.ascontiguousarray(xs.T).astype(BF16)  # [512, 1152]
        xa = np.ascontiguousarray(xT.reshape(4, 128, TOKP).transpose(1, 0, 2))
        cstc = cstv.copy()
        cstc[:, 8] = 1.0 if lo >= 0 else 0.0
        cstc[:, 9] = 1.0 if hi <= S else 0.0
        in_maps.append({"xa": xa, "w1": w1a, "cst": cstc, **wq_common})
    return in_maps


_NC_CACHE = {}


def get_nc():
    if "nc" not in _NC_CACHE:
        _NC_CACHE["nc"] = build_nc()
    return _NC_CACHE["nc"]


def kernel(x, W1, b1, Wq, atten_size, _trace=False, _trace_kwargs=None):
    assert int(atten_size) == A, f"kernel hardcodes atten_size=16, got {atten_size}"
    nc = get_nc()
    in_maps = make_inputs(x, W1, b1, Wq)
    kw = {}
    if _trace:
        kw = dict(trace=True, trace_kwargs=_trace_kwargs or {})
    res = run_bass_kernel_spmd(nc, in_maps, core_ids=list(range(NCORES)), **kw)
    out = np.stack([r["out"].astype(np.float32) for r in res.results])
    out = out.reshape(B, S, H)
    if _trace:
        return out, res
    return out


if __name__ == "__main__":
    import jax

    key = jax.random.key(0)
    k1, k2, k3, k4 = jax.random.split(key, 4)
    x = np.asarray(jax.random.normal(k1, (B, S, IN), dtype=np.float32))
    W1 = np.asarray(
        jax.random.normal(k2, (IN, H), dtype=np.float32) * (1.0 / np.sqrt(IN))
    )
    b1 = np.asarray(jax.random.normal(k3, (H,), dtype=np.float32) * 0.02)
    Wq = np.asarray(
        jax.random.normal(k4, (H, H), dtype=np.float32) * (1.0 / np.sqrt(H))
    )
    out = kernel(x, W1, b1, Wq, 16)
    print("out", out.shape, out.dtype, float(np.abs(out).max()))
